# revision 16
# baseline (speedup 1.0000x reference)
"""Bond-aware message passing GNN kernel for 8 Trainium2 NeuronCores.

Strategy (edge-parallel, col-sorted):
  - Host: sort edges by destination (col), shard contiguous ranges of sorted
    edges across 8 cores, pack per-512-edge-tile inputs in feature-major
    (transposed) layout so the device only does W-stationary matmuls.
  - Device per 512-edge tile:
      L1:  h_m = W1_m.T @ combinedT  (3 MLPs, K=146 split 128+18; the 18th
           lo-row is a constant 1 whose weight row is b1 -> bias folded in)
      act: h = silu(h) in one ScalarE op over all 3 MLPs
      L2:  msgT = W2cat.T @ h        (3 matmuls into one PSUM tile)
      +b2 and move to SBUF (DVE), PE-transpose payload to edge-major,
      selection-matrix scatter matmul (segment-sum within the tile's
      64-node window), window results stacked to DRAM.
  - Host: overlap-add the per-tile 64-node windows into the final
    aggregated_x / aggregated_pos; inverse-permute edge_update.

No collectives: cores own disjoint edge ranges; window overlap across
tile/core boundaries is resolved in the host merge.
"""

import contextlib

import numpy as np

import concourse.bass as bass
from concourse import bacc
import concourse.mybir as mybir
import concourse.tile as tile
from concourse import bass_utils
from concourse.masks import make_identity

F32 = mybir.dt.float32
I32 = mybir.dt.int32

# model dims (fixed by the problem)
IN_D, BOND_D, HID, OUT_D = 64, 16, 128, 64
COMB = 2 * IN_D + BOND_D + 1  # 145
KLO = COMB - 128 + 1  # 17 real rows + ones-row for b1
NCORES = 8
TILE = 512  # edges per device tile
SUB = 128  # edges per subtile (matmul K)
WIN = 64  # node-window slots per tile
PAY = 67  # payload rows: 64 msg_x + 3 pos_update
# P2 psum row layout: [0:64] msg_x, [64:67] w_pos(x3), [96:112] edge_update
EU_LO, EU_HI = 96, 112
GRP = 4  # tiles per output-batching group
# per-partition f32 layout of the main input stream: chi | rel | colw
MAIN_W = TILE + 12 + 4  # 528

LAST_RESULTS = None  # BassKernelResults of the most recent run (for test.py)
ACT_FUNC = mybir.ActivationFunctionType.Silu  # overridable for CoreSim tests


# ----------------------------------------------------------------------------
# host-side packing
# ----------------------------------------------------------------------------

def _plan_tiles(col_s, lo, hi):
    """Greedy tiling of sorted edge range [lo,hi): each tile takes <=TILE edges
    whose cols stay within a WIN-node window. Returns list of (pos, take, base)."""
    tiles = []
    pos = lo
    while pos < hi:
        base = int(col_s[pos])
        end = min(pos + TILE, hi)
        take = int(np.searchsorted(col_s[pos:end], base + WIN, side="left"))
        assert take > 0
        tiles.append((pos, take, base))
        pos += take
    return tiles


def _pack_core(x, pos_arr, ea_s, row_s, col_s, rel_s, dsq_s, lo, hi, T):
    """Build device input arrays for one core's sorted-edge range [lo, hi)."""
    tiles = _plan_tiles(col_s, lo, hi)
    assert len(tiles) <= T

    # per-slot source index into the sorted arrays (-1 = padding)
    idx = np.full((T, TILE), -1, dtype=np.int64)
    bases = np.zeros((T,), dtype=np.int64)
    for t, (p, take, b) in enumerate(tiles):
        idx[t, :take] = np.arange(p, p + take)
        bases[t] = b
    valid = idx >= 0
    cidx = np.where(valid, idx, 0)

    r = np.where(valid, row_s[cidx], 0)
    c = np.where(valid, col_s[cidx], 0)

    xr = np.where(valid[..., None], x[r], 0.0)  # [T, TILE, 64]
    xc = np.where(valid[..., None], x[c], 0.0)
    ea = np.where(valid[..., None], ea_s[cidx], 0.0)  # [T, TILE, 16]
    rel = np.where(valid[..., None], rel_s[cidx], 0.0)  # [T, TILE, 3]
    dsq = np.where(valid, dsq_s[cidx], 0.0)  # [T, TILE]

    nsub = TILE // SUB
    main = np.zeros((T, 128, MAIN_W), np.float32)
    main[:, 0:64, 0:TILE] = xr.transpose(0, 2, 1)
    main[:, 64:128, 0:TILE] = xc.transpose(0, 2, 1)
    # rel edge-major [T, 128, nsub, 3]
    main[:, :, TILE : TILE + 12] = (
        rel.reshape(T, nsub, SUB, 3).transpose(0, 2, 1, 3).reshape(T, 128, 12)
    )
    col_local = np.where(valid, c - bases[:, None], -1).astype(np.float32)
    main[:, :, TILE + 12 : MAIN_W] = col_local.reshape(T, nsub, SUB).transpose(
        0, 2, 1
    )

    comb_lo = np.concatenate(
        [ea.transpose(0, 2, 1), dsq[:, None, :], np.ones((T, 1, TILE))], axis=1
    ).astype(np.float32)  # [T, 18, TILE]

    return {
        "main": main,
        "comb_lo": np.ascontiguousarray(comb_lo),
    }, idx, bases


def _pack_weights(W1x, W1p, W1e, b1x, b1p, b1e, W2x, W2p, W2e, b2x, b2p, b2e):
    W1hi = np.stack([W1x[:128], W1p[:128], W1e[:128]], axis=1)  # [128, 3, HID]
    # lo rows + b1 as the ones-row weight
    W1lo = np.stack(
        [np.concatenate([W1x[128:], b1x[None, :]], axis=0),
         np.concatenate([W1p[128:], b1p[None, :]], axis=0),
         np.concatenate([W1e[128:], b1e[None, :]], axis=0)], axis=1
    )  # [18, 3, HID]
    W2cat = np.concatenate(
        [W2x, np.repeat(W2p, 3, axis=1), W2e], axis=1
    )  # [HID, 64+3+16=83]
    b2cat = np.zeros((EU_HI, 1), np.float32)
    b2cat[0:64, 0] = b2x
    b2cat[64:67, 0] = b2p
    b2cat[EU_LO:EU_HI, 0] = b2e
    return {
        "W1hi": np.ascontiguousarray(W1hi, np.float32),
        "W1lo": np.ascontiguousarray(W1lo, np.float32),
        "W2cat": np.ascontiguousarray(W2cat, np.float32),
        "b2cat": b2cat,
    }


# ----------------------------------------------------------------------------
# device program
# ----------------------------------------------------------------------------

def _build_bass(T, reps=1):
    nc = bacc.Bacc(trn_type="TRN2")
    nsub = TILE // SUB
    assert T % GRP == 0

    d_main = nc.dram_tensor("main", [T, 128, MAIN_W], F32, kind="ExternalInput")
    d_clo = nc.dram_tensor("comb_lo", [T, KLO, TILE], F32, kind="ExternalInput")
    d_W1hi = nc.dram_tensor("W1hi", [128, 3, HID], F32, kind="ExternalInput")
    d_W1lo = nc.dram_tensor("W1lo", [KLO, 3, HID], F32, kind="ExternalInput")
    d_W2cat = nc.dram_tensor("W2cat", [HID, 83], F32, kind="ExternalInput")
    d_b2cat = nc.dram_tensor("b2cat", [EU_HI, 1], F32, kind="ExternalInput")

    d_win = nc.dram_tensor("winstack", [WIN, T * PAY], F32, kind="ExternalOutput")
    d_eu = nc.dram_tensor("euT", [BOND_D, T * TILE], F32, kind="ExternalOutput")

    with tile.TileContext(nc) as tc:
        with (
            tc.tile_pool(name="const", bufs=1) as constp,
            tc.tile_pool(name="io", bufs=3) as iop,
            tc.tile_pool(name="work", bufs=2) as workp,
            tc.tile_pool(name="eusb", bufs=2) as eup,
            tc.tile_pool(name="winsb", bufs=1) as winp,
            tc.tile_pool(name="ph", bufs=2, space="PSUM") as php,
            tc.tile_pool(name="p2", bufs=1, space="PSUM") as p2p,
            tc.tile_pool(name="ptr", bufs=1, space="PSUM") as ptrp,
        ):
            # ---- constants / weights (loaded once) ----
            ident = constp.tile([128, 128], F32)
            make_identity(nc, ident[:])

            iota_i = constp.tile([128, nsub, WIN], I32)
            nc.gpsimd.iota(iota_i[:], pattern=[[0, nsub], [1, WIN]], base=0,
                           channel_multiplier=0)
            iota_f = constp.tile([128, nsub, WIN], F32)
            nc.vector.tensor_copy(iota_f[:], iota_i[:])

            w1hi = constp.tile([128, 3, HID], F32)
            nc.sync.dma_start(out=w1hi[:], in_=d_W1hi[:])
            w1lo = constp.tile([KLO, 3, HID], F32)
            nc.sync.dma_start(out=w1lo[:], in_=d_W1lo[:])
            w2cat = constp.tile([HID, 83], F32)
            nc.sync.dma_start(out=w2cat[:], in_=d_W2cat[:])
            b2cat = constp.tile([EU_HI, 1], F32)
            nc.sync.dma_start(out=b2cat[:], in_=d_b2cat[:])

            # winstack staging buffer in SBUF (one DMA at the end); slot-major
            win_sb = winp.tile([WIN, T * PAY], F32)

            rep_ctx = tc.For_i(0, reps, 1) if reps > 1 else contextlib.nullcontext()
            with rep_ctx:
                for g in range(T // GRP):
                    eu_sb = eup.tile([BOND_D, GRP, TILE], F32, tag="eusb")
                    agg = ptrp.tile([WIN, GRP, PAY], F32, tag="agg")
                    for ti in range(GRP):
                        t = g * GRP + ti
                        main = iop.tile([128, MAIN_W], F32, tag="main")
                        nc.sync.dma_start(out=main[:], in_=d_main[t])
                        clo = iop.tile([KLO, TILE], F32, tag="clo")
                        nc.sync.dma_start(out=clo[:], in_=d_clo[t])
                        chi = main[:, 0:TILE]
                        rel = main[:, TILE : TILE + 12].rearrange(
                            "p (s k) -> p s k", k=3
                        )
                        colw = main[:, TILE + 12 : MAIN_W]

                        # ---- L1: h[m] = W1_m.T @ combT (b1 folded in lo) ----
                        h_ps = php.tile([128, 3, TILE], F32, tag="hps")
                        for m_ in range(3):
                            nc.tensor.matmul(
                                h_ps[:, m_, :], w1hi[:, m_, :], chi,
                                start=True, stop=False,
                            )
                            nc.tensor.matmul(
                                h_ps[:, m_, :], w1lo[:, m_, :], clo[:],
                                start=False, stop=True,
                            )

                        # ---- silu -> SBUF, one op over all 3 MLPs ----
                        h_sb = workp.tile([128, 3, TILE], F32, tag="hsb")
                        nc.scalar.activation(h_sb[:], h_ps[:], ACT_FUNC)

                        # ---- L2 into one PSUM tile ----
                        p2 = p2p.tile([128, TILE], F32, tag="p2")
                        nc.tensor.matmul(p2[0:64, :], w2cat[:, 0:64],
                                         h_sb[:, 0, :], start=True, stop=True)
                        nc.tensor.matmul(p2[64:67, :], w2cat[:, 64:67],
                                         h_sb[:, 1, :], start=True, stop=True,
                                         tile_position=(0, 64))
                        nc.tensor.matmul(p2[EU_LO:EU_HI, :], w2cat[:, 67:83],
                                         h_sb[:, 2, :], start=True, stop=True,
                                         tile_position=(0, EU_LO))

                        # ---- +b2, move to SBUF ----
                        payT = workp.tile([EU_HI, TILE], F32, tag="payT")
                        nc.vector.tensor_scalar(
                            out=payT[0:PAY, :], in0=p2[0:PAY, :],
                            scalar1=b2cat[0:PAY, 0:1], scalar2=None,
                            op0=mybir.AluOpType.add,
                        )
                        nc.vector.tensor_scalar(
                            out=eu_sb[:, ti, :], in0=p2[EU_LO:EU_HI, :],
                            scalar1=b2cat[EU_LO:EU_HI, 0:1], scalar2=None,
                            op0=mybir.AluOpType.add,
                        )

                        # ---- transpose payload to edge-major ----
                        # reuses the p2 bank: L2 results are dead once the
                        # bias moves complete (WAR deps order this correctly)
                        for s in range(nsub):
                            nc.tensor.transpose(
                                p2[:, s * PAY : (s + 1) * PAY],
                                payT[0:PAY, s * SUB : (s + 1) * SUB],
                                ident[0:PAY, 0:PAY],
                            )
                        pay_em = workp.tile([128, nsub, PAY], F32, tag="payem")
                        nc.vector.tensor_copy(
                            pay_em[:],
                            p2[:, 0 : nsub * PAY].rearrange(
                                "p (s f) -> p s f", f=PAY
                            ),
                        )
                        # pos_update = w_pos * rel_pos
                        nc.vector.tensor_tensor(
                            out=pay_em[:, :, 64:67], in0=pay_em[:, :, 64:67],
                            in1=rel, op=mybir.AluOpType.mult,
                        )

                        # ---- S[e, slot] = (col_local[e] == slot) ----
                        S = workp.tile([128, nsub, WIN], F32, tag="S")
                        nc.vector.tensor_tensor(
                            out=S[:], in0=iota_f[:],
                            in1=colw[:, :, None].to_broadcast([128, nsub, WIN]),
                            op=mybir.AluOpType.is_equal,
                        )

                        # ---- scatter: agg[slot, ti, feat] += S_s.T @ pay_s ----
                        for s in range(nsub):
                            nc.tensor.matmul(
                                agg[:, ti, :], S[:, s, :], pay_em[:, s, :],
                                start=(s == 0), stop=(s == nsub - 1),
                            )

                    # ---- batched outputs for the group ----
                    nc.vector.tensor_copy(
                        win_sb[:, g * GRP * PAY : (g + 1) * GRP * PAY], agg[:]
                    )
                    nc.sync.dma_start(
                        out=d_eu[:, g * GRP * TILE : (g + 1) * GRP * TILE],
                        in_=eu_sb[:],
                    )

            nc.sync.dma_start(out=d_win[:], in_=win_sb[:])

    nc.finalize()
    return nc


# ----------------------------------------------------------------------------
# entry point
# ----------------------------------------------------------------------------

def host_pack(x, pos, edge_attr, edge_index,
              W1x, b1x, W2x, b2x,
              W1p, b1p, W2p, b2p,
              W1e, b1e, W2e, b2e):
    """Sort/shard/pack inputs. Returns (core_inputs, meta)."""
    x = np.asarray(x, np.float32)
    pos = np.asarray(pos, np.float32)
    edge_attr = np.asarray(edge_attr, np.float32)
    edge_index = np.asarray(edge_index)
    E = edge_index.shape[1]
    N = x.shape[0]

    row = edge_index[0].astype(np.int64)
    col = edge_index[1].astype(np.int64)
    order = np.argsort(col, kind="stable")
    row_s = row[order]
    col_s = col[order]
    ea_s = edge_attr[order]
    rel_s = pos[row_s] - pos[col_s]  # [E, 3]
    dsq_s = np.sum(rel_s * rel_s, axis=1)  # [E]

    # shard sorted edges evenly
    bounds = [E * c // NCORES for c in range(NCORES + 1)]
    plans = [
        _plan_tiles(col_s, bounds[c], bounds[c + 1]) for c in range(NCORES)
    ]
    T = max(len(p) for p in plans)
    T = (T + GRP - 1) // GRP * GRP

    core_inputs = []
    core_idx = []
    core_bases = []
    wts = _pack_weights(W1x, W1p, W1e, b1x, b1p, b1e,
                        W2x, W2p, W2e, b2x, b2p, b2e)
    for c in range(NCORES):
        inp, idx, bases = _pack_core(
            x, pos, ea_s, row_s, col_s, rel_s, dsq_s,
            bounds[c], bounds[c + 1], T,
        )
        inp.update(wts)
        core_inputs.append(inp)
        core_idx.append(idx)
        core_bases.append(bases)
    meta = {
        "N": N, "E": E, "T": T, "order": order, "plans": plans,
        "core_idx": core_idx, "core_bases": core_bases,
    }
    return core_inputs, meta


def host_merge(results, meta):
    N, E, T = meta["N"], meta["E"], meta["T"]
    order, plans = meta["order"], meta["plans"]
    core_idx, core_bases = meta["core_idx"], meta["core_bases"]

    aggX = np.zeros((N + WIN, OUT_D), np.float64)
    aggP = np.zeros((N + WIN, 3), np.float64)
    edge_update_s = np.empty((E, BOND_D), np.float32)
    for c in range(NCORES):
        out = results[c]
        win = out["winstack"].reshape(WIN, T, PAY)  # [slot, tile, feat]
        euT = out["euT"]  # [16, T*TILE]
        idx = core_idx[c]
        bases = core_bases[c]
        nt = len(plans[c])
        # overlap-add the windows
        wx = win[:, :nt, 0:64].transpose(1, 0, 2)  # [nt, WIN, 64]
        wpp = win[:, :nt, 64:67].transpose(1, 0, 2)  # [nt, WIN, 3]
        tgt = (bases[:nt, None] + np.arange(WIN)[None, :]).ravel()
        np.add.at(aggX, tgt, wx.reshape(-1, OUT_D))
        np.add.at(aggP, tgt, wpp.reshape(-1, 3))
        # edge updates back to sorted order
        eu = euT.T.reshape(T, TILE, BOND_D)
        v = idx >= 0
        edge_update_s[idx[v]] = eu[v]

    aggregated_x = aggX[:N].astype(np.float32)
    aggregated_pos = aggP[:N].astype(np.float32)
    edge_update = np.empty_like(edge_update_s)
    edge_update[order] = edge_update_s
    return aggregated_x, aggregated_pos, edge_update


def kernel(**inputs):
    global LAST_RESULTS
    core_inputs, meta = host_pack(**inputs)
    nc = _build_bass(meta["T"])
    res = bass_utils.run_bass_kernel_spmd(
        nc, core_inputs, core_ids=list(range(NCORES)),
    )
    LAST_RESULTS = res
    return host_merge(res.results, meta)


# revision 21
# speedup vs baseline: 1.3943x; 1.3943x over previous
"""Bond-aware message passing GNN kernel for 8 Trainium2 NeuronCores.

Strategy (edge-parallel, col-sorted):
  - Host: sort edges by destination (col), shard contiguous ranges of sorted
    edges across 8 cores, pack per-512-edge-tile inputs in feature-major
    (transposed) layout so the device only does W-stationary matmuls.
  - Device per 512-edge tile:
      L1:  h_m = W1_m.T @ combinedT  (3 MLPs, K=146 split 128+18; the 18th
           lo-row is a constant 1 whose weight row is b1 -> bias folded in)
      act: h = silu(h) in one ScalarE op over all 3 MLPs
      L2:  msgT = W2cat.T @ h        (3 matmuls into one PSUM tile)
      +b2 and move to SBUF (DVE), PE-transpose payload to edge-major,
      selection-matrix scatter matmul (segment-sum within the tile's
      64-node window), window results stacked to DRAM.
  - Host: overlap-add the per-tile 64-node windows into the final
    aggregated_x / aggregated_pos; inverse-permute edge_update.

No collectives: cores own disjoint edge ranges; window overlap across
tile/core boundaries is resolved in the host merge.
"""

import contextlib

import numpy as np

import concourse.bass as bass
from concourse import bacc
import concourse.mybir as mybir
import concourse.tile as tile
from concourse import bass_utils
from concourse.masks import make_identity

F32 = mybir.dt.float32
F32R = mybir.dt.float32r
I32 = mybir.dt.int32


def _r(ap):
    return ap.bitcast(F32R)

# model dims (fixed by the problem)
IN_D, BOND_D, HID, OUT_D = 64, 16, 128, 64
COMB = 2 * IN_D + BOND_D + 1  # 145
KLO = COMB - 128 + 1  # 17 real rows + ones-row for b1
NCORES = 8
TILE = 512  # edges per device tile
SUB = 128  # edges per subtile (matmul K)
WIN = 64  # node-window slots per tile
PAY = 67  # payload rows: 64 msg_x + 3 pos_update
PAYP = 68  # padded (even) transposed-payload width for fp32r matmuls
# P2 psum row layout: [0:64] msg_x, [64:67] w_pos(x3), [67:83] edge_update
EU_LO, EU_HI = 67, 83
P2H = 84  # L2 output rows (83 + 1 even-pad)
GRP = 4  # tiles per output-batching group
# per-partition f32 layout of the main input stream: chi | rel | colw
MAIN_W = TILE + 12 + 4  # 528

LAST_RESULTS = None  # BassKernelResults of the most recent run (for test.py)
ACT_FUNC = mybir.ActivationFunctionType.Silu  # overridable for CoreSim tests


# ----------------------------------------------------------------------------
# host-side packing
# ----------------------------------------------------------------------------

def _plan_tiles(col_s, lo, hi):
    """Greedy tiling of sorted edge range [lo,hi): each tile takes <=TILE edges
    whose cols stay within a WIN-node window. Returns list of (pos, take, base)."""
    tiles = []
    pos = lo
    while pos < hi:
        base = int(col_s[pos])
        end = min(pos + TILE, hi)
        take = int(np.searchsorted(col_s[pos:end], base + WIN, side="left"))
        assert take > 0
        tiles.append((pos, take, base))
        pos += take
    return tiles


def _pack_core(x, pos_arr, ea_s, row_s, col_s, rel_s, dsq_s, lo, hi, T):
    """Build device input arrays for one core's sorted-edge range [lo, hi)."""
    tiles = _plan_tiles(col_s, lo, hi)
    assert len(tiles) <= T

    # per-slot source index into the sorted arrays (-1 = padding)
    idx = np.full((T, TILE), -1, dtype=np.int64)
    bases = np.zeros((T,), dtype=np.int64)
    for t, (p, take, b) in enumerate(tiles):
        idx[t, :take] = np.arange(p, p + take)
        bases[t] = b
    valid = idx >= 0
    cidx = np.where(valid, idx, 0)

    r = np.where(valid, row_s[cidx], 0)
    c = np.where(valid, col_s[cidx], 0)

    xr = np.where(valid[..., None], x[r], 0.0)  # [T, TILE, 64]
    xc = np.where(valid[..., None], x[c], 0.0)
    ea = np.where(valid[..., None], ea_s[cidx], 0.0)  # [T, TILE, 16]
    rel = np.where(valid[..., None], rel_s[cidx], 0.0)  # [T, TILE, 3]
    dsq = np.where(valid, dsq_s[cidx], 0.0)  # [T, TILE]

    nsub = TILE // SUB
    main = np.zeros((T, 128, MAIN_W), np.float32)
    main[:, 0:64, 0:TILE] = xr.transpose(0, 2, 1)
    main[:, 64:128, 0:TILE] = xc.transpose(0, 2, 1)
    # rel edge-major [T, 128, nsub, 3]
    main[:, :, TILE : TILE + 12] = (
        rel.reshape(T, nsub, SUB, 3).transpose(0, 2, 1, 3).reshape(T, 128, 12)
    )
    col_local = np.where(valid, c - bases[:, None], -1).astype(np.float32)
    main[:, :, TILE + 12 : MAIN_W] = col_local.reshape(T, nsub, SUB).transpose(
        0, 2, 1
    )

    comb_lo = np.concatenate(
        [ea.transpose(0, 2, 1), dsq[:, None, :], np.ones((T, 1, TILE))], axis=1
    ).astype(np.float32)  # [T, 18, TILE]

    return {
        "main": main,
        "comb_lo": np.ascontiguousarray(comb_lo),
    }, idx, bases


def _pack_weights(W1x, W1p, W1e, b1x, b1p, b1e, W2x, W2p, W2e, b2x, b2p, b2e):
    W1hi = np.stack([W1x[:128], W1p[:128], W1e[:128]], axis=1)  # [128, 3, HID]
    # lo rows + b1 as the ones-row weight
    W1lo = np.stack(
        [np.concatenate([W1x[128:], b1x[None, :]], axis=0),
         np.concatenate([W1p[128:], b1p[None, :]], axis=0),
         np.concatenate([W1e[128:], b1e[None, :]], axis=0)], axis=1
    )  # [18, 3, HID]
    # block-diagonal second layer: one K=384 accumulation into [84, 512]
    W2blk = np.zeros((128, 3, P2H), np.float32)
    W2blk[:, 0, 0:64] = W2x
    W2blk[:, 1, 64:67] = np.repeat(W2p, 3, axis=1)
    W2blk[:, 2, EU_LO:EU_HI] = W2e
    b2cat = np.zeros((P2H, 1), np.float32)
    b2cat[0:64, 0] = b2x
    b2cat[64:67, 0] = b2p
    b2cat[EU_LO:EU_HI, 0] = b2e
    return {
        "W1hi": np.ascontiguousarray(W1hi, np.float32),
        "W1lo": np.ascontiguousarray(W1lo, np.float32),
        "W2blk": W2blk,
        "b2cat": b2cat,
    }


# ----------------------------------------------------------------------------
# device program
# ----------------------------------------------------------------------------

def _build_bass(T, reps=1):
    nc = bacc.Bacc(trn_type="TRN2")
    nsub = TILE // SUB
    assert T % GRP == 0

    d_main = nc.dram_tensor("main", [T, 128, MAIN_W], F32R, kind="ExternalInput")
    d_clo = nc.dram_tensor("comb_lo", [T, KLO, TILE], F32R, kind="ExternalInput")
    d_W1hi = nc.dram_tensor("W1hi", [128, 3, HID], F32R, kind="ExternalInput")
    d_W1lo = nc.dram_tensor("W1lo", [KLO, 3, HID], F32R, kind="ExternalInput")
    d_W2blk = nc.dram_tensor("W2blk", [HID, 3, P2H], F32R, kind="ExternalInput")
    d_b2cat = nc.dram_tensor("b2cat", [P2H, 1], F32, kind="ExternalInput")

    d_win = nc.dram_tensor("winstack", [WIN, T * PAY], F32, kind="ExternalOutput")
    d_eu = nc.dram_tensor("euT", [BOND_D, T * TILE], F32, kind="ExternalOutput")

    with tile.TileContext(nc) as tc:
        with (
            tc.tile_pool(name="const", bufs=1) as constp,
            tc.tile_pool(name="io", bufs=3) as iop,
            tc.tile_pool(name="work", bufs=2) as workp,
            tc.tile_pool(name="winsb", bufs=1) as winp,
            tc.tile_pool(name="ph", bufs=2, space="PSUM") as php,
            tc.tile_pool(name="p2", bufs=1, space="PSUM") as p2p,
            tc.tile_pool(name="ptr", bufs=1, space="PSUM") as ptrp,
        ):
            # ---- constants / weights (loaded once) ----
            ident = constp.tile([128, 128], F32)
            make_identity(nc, ident[:])

            iota_i = constp.tile([128, nsub, WIN], I32)
            nc.gpsimd.iota(iota_i[:], pattern=[[0, nsub], [1, WIN]], base=0,
                           channel_multiplier=0)
            iota_f = constp.tile([128, nsub, WIN], F32R)
            nc.vector.tensor_copy(iota_f[:], iota_i[:])

            w1hi = constp.tile([128, 3, HID], F32R)
            nc.sync.dma_start(out=w1hi[:], in_=d_W1hi[:])
            w1lo = constp.tile([KLO, 3, HID], F32R)
            nc.sync.dma_start(out=w1lo[:], in_=d_W1lo[:])
            w2blk = constp.tile([HID, 3, P2H], F32R)
            nc.sync.dma_start(out=w2blk[:], in_=d_W2blk[:])
            b2cat = constp.tile([P2H, 1], F32)
            nc.sync.dma_start(out=b2cat[:], in_=d_b2cat[:])

            # winstack staging buffer in SBUF (one DMA at the end); slot-major
            win_sb = winp.tile([WIN, T * PAY], F32)

            rep_ctx = tc.For_i(0, reps, 1) if reps > 1 else contextlib.nullcontext()
            with rep_ctx:
                for g in range(T // GRP):
                    agg = ptrp.tile([WIN, GRP, PAYP], F32, tag="agg")
                    for ti in range(GRP):
                        t = g * GRP + ti
                        main = iop.tile([128, MAIN_W], F32R, tag="main")
                        nc.sync.dma_start(out=main[:], in_=d_main[t])
                        clo = iop.tile([KLO, TILE], F32R, tag="clo")
                        nc.sync.dma_start(out=clo[:], in_=d_clo[t])
                        chi = main[:, 0:TILE]
                        rel = main[:, TILE : TILE + 12].rearrange(
                            "p (s k) -> p s k", k=3
                        )
                        colw = main[:, TILE + 12 : MAIN_W]

                        # ---- L1: h[m] = W1_m.T @ combT (b1 folded in lo) ----
                        h_ps = php.tile([128, 3, TILE], F32, tag="hps")
                        for m_ in range(3):
                            nc.tensor.matmul(
                                h_ps[:, m_, :], w1hi[:, m_, :], chi,
                                start=True, stop=False,
                            )
                            nc.tensor.matmul(
                                h_ps[:, m_, :], w1lo[:, m_, :], clo[:],
                                start=False, stop=True,
                            )

                        # ---- silu -> SBUF, one op over all 3 MLPs ----
                        h_sb = workp.tile([128, 3, TILE], F32R, tag="hsb")
                        nc.scalar.activation(h_sb[:], h_ps[:], ACT_FUNC)

                        # ---- L2: one K=384 block-diag accumulation ----
                        p2 = p2p.tile([128, TILE], F32, tag="p2")
                        for c in range(3):
                            nc.tensor.matmul(
                                p2[0:P2H, :], w2blk[:, c, :], h_sb[:, c, :],
                                start=(c == 0), stop=(c == 2),
                            )

                        # ---- +b2, move to SBUF (one op) ----
                        payT = workp.tile([P2H, TILE], F32, tag="payT")
                        nc.vector.tensor_scalar(
                            out=payT[:], in0=p2[0:P2H, :],
                            scalar1=b2cat[:, 0:1], scalar2=None,
                            op0=mybir.AluOpType.add,
                        )
                        nc.sync.dma_start(
                            out=d_eu[:, t * TILE : (t + 1) * TILE],
                            in_=payT[EU_LO:EU_HI, :],
                        )

                        # ---- transpose payload to edge-major ----
                        # reuses the p2 bank: L2 results are dead once the
                        # bias moves complete (WAR deps order this correctly)
                        for s in range(nsub):
                            nc.tensor.transpose(
                                p2[:, s * PAYP : (s + 1) * PAYP],
                                payT[0:PAYP, s * SUB : (s + 1) * SUB],
                                ident[0:PAYP, 0:PAYP],
                            )
                        pay_em = workp.tile([128, nsub, PAYP], F32R, tag="payem")
                        nc.vector.tensor_copy(
                            pay_em[:],
                            p2[:, 0 : nsub * PAYP].rearrange(
                                "p (s f) -> p s f", f=PAYP
                            ),
                        )
                        # pos_update = w_pos * rel_pos
                        nc.vector.tensor_tensor(
                            out=pay_em[:, :, 64:67], in0=pay_em[:, :, 64:67],
                            in1=rel, op=mybir.AluOpType.mult,
                        )

                        # ---- S[e, slot] = (col_local[e] == slot) ----
                        S = workp.tile([128, nsub, WIN], F32R, tag="S")
                        nc.vector.tensor_tensor(
                            out=S[:], in0=iota_f[:],
                            in1=colw[:, :, None].to_broadcast([128, nsub, WIN]),
                            op=mybir.AluOpType.is_equal,
                        )

                        # ---- scatter: agg[slot, ti, feat] += S_s.T @ pay_s ----
                        for s in range(nsub):
                            nc.tensor.matmul(
                                agg[:, ti, :], S[:, s, :], pay_em[:, s, :],
                                start=(s == 0), stop=(s == nsub - 1),
                            )

                    # ---- batched outputs for the group ----
                    nc.vector.tensor_copy(
                        win_sb[:, g * GRP * PAY : (g + 1) * GRP * PAY],
                        agg[:, :, 0:PAY],
                    )

            nc.sync.dma_start(out=d_win[:], in_=win_sb[:])

    nc.finalize()
    return nc


# ----------------------------------------------------------------------------
# entry point
# ----------------------------------------------------------------------------

def host_pack(x, pos, edge_attr, edge_index,
              W1x, b1x, W2x, b2x,
              W1p, b1p, W2p, b2p,
              W1e, b1e, W2e, b2e):
    """Sort/shard/pack inputs. Returns (core_inputs, meta)."""
    x = np.asarray(x, np.float32)
    pos = np.asarray(pos, np.float32)
    edge_attr = np.asarray(edge_attr, np.float32)
    edge_index = np.asarray(edge_index)
    E = edge_index.shape[1]
    N = x.shape[0]

    row = edge_index[0].astype(np.int64)
    col = edge_index[1].astype(np.int64)
    order = np.argsort(col, kind="stable")
    row_s = row[order]
    col_s = col[order]
    ea_s = edge_attr[order]
    rel_s = pos[row_s] - pos[col_s]  # [E, 3]
    dsq_s = np.sum(rel_s * rel_s, axis=1)  # [E]

    # shard sorted edges evenly
    bounds = [E * c // NCORES for c in range(NCORES + 1)]
    plans = [
        _plan_tiles(col_s, bounds[c], bounds[c + 1]) for c in range(NCORES)
    ]
    T = max(len(p) for p in plans)
    T = (T + GRP - 1) // GRP * GRP

    core_inputs = []
    core_idx = []
    core_bases = []
    wts = _pack_weights(W1x, W1p, W1e, b1x, b1p, b1e,
                        W2x, W2p, W2e, b2x, b2p, b2e)
    for c in range(NCORES):
        inp, idx, bases = _pack_core(
            x, pos, ea_s, row_s, col_s, rel_s, dsq_s,
            bounds[c], bounds[c + 1], T,
        )
        inp.update(wts)
        core_inputs.append(inp)
        core_idx.append(idx)
        core_bases.append(bases)
    meta = {
        "N": N, "E": E, "T": T, "order": order, "plans": plans,
        "core_idx": core_idx, "core_bases": core_bases,
    }
    return core_inputs, meta


def host_merge(results, meta):
    N, E, T = meta["N"], meta["E"], meta["T"]
    order, plans = meta["order"], meta["plans"]
    core_idx, core_bases = meta["core_idx"], meta["core_bases"]

    aggX = np.zeros((N + WIN, OUT_D), np.float64)
    aggP = np.zeros((N + WIN, 3), np.float64)
    edge_update_s = np.empty((E, BOND_D), np.float32)
    for c in range(NCORES):
        out = results[c]
        win = out["winstack"].reshape(WIN, T, PAY)  # [slot, tile, feat]
        euT = out["euT"]  # [16, T*TILE]
        idx = core_idx[c]
        bases = core_bases[c]
        nt = len(plans[c])
        # overlap-add the windows
        wx = win[:, :nt, 0:64].transpose(1, 0, 2)  # [nt, WIN, 64]
        wpp = win[:, :nt, 64:67].transpose(1, 0, 2)  # [nt, WIN, 3]
        tgt = (bases[:nt, None] + np.arange(WIN)[None, :]).ravel()
        np.add.at(aggX, tgt, wx.reshape(-1, OUT_D))
        np.add.at(aggP, tgt, wpp.reshape(-1, 3))
        # edge updates back to sorted order
        eu = euT.T.reshape(T, TILE, BOND_D)
        v = idx >= 0
        edge_update_s[idx[v]] = eu[v]

    aggregated_x = aggX[:N].astype(np.float32)
    aggregated_pos = aggP[:N].astype(np.float32)
    edge_update = np.empty_like(edge_update_s)
    edge_update[order] = edge_update_s
    return aggregated_x, aggregated_pos, edge_update


def kernel(**inputs):
    global LAST_RESULTS
    core_inputs, meta = host_pack(**inputs)
    nc = _build_bass(meta["T"])
    res = bass_utils.run_bass_kernel_spmd(
        nc, core_inputs, core_ids=list(range(NCORES)),
    )
    LAST_RESULTS = res
    return host_merge(res.results, meta)


# revision 23
# speedup vs baseline: 1.5732x; 1.1284x over previous
"""Bond-aware message passing GNN kernel for 8 Trainium2 NeuronCores.

Strategy (edge-parallel, col-sorted):
  - Host: sort edges by destination (col), shard contiguous ranges of sorted
    edges across 8 cores, pack per-512-edge-tile inputs in feature-major
    (transposed) layout so the device only does W-stationary matmuls.
  - Device per 512-edge tile:
      L1:  h_m = W1_m.T @ combinedT  (3 MLPs, K=146 split 128+18; the 18th
           lo-row is a constant 1 whose weight row is b1 -> bias folded in)
      act: h = silu(h) in one ScalarE op over all 3 MLPs
      L2:  msgT = W2cat.T @ h        (3 matmuls into one PSUM tile)
      +b2 and move to SBUF (DVE), PE-transpose payload to edge-major,
      selection-matrix scatter matmul (segment-sum within the tile's
      64-node window), window results stacked to DRAM.
  - Host: overlap-add the per-tile 64-node windows into the final
    aggregated_x / aggregated_pos; inverse-permute edge_update.

No collectives: cores own disjoint edge ranges; window overlap across
tile/core boundaries is resolved in the host merge.
"""

import contextlib

import numpy as np

import concourse.bass as bass
from concourse import bacc
import concourse.mybir as mybir
import concourse.tile as tile
from concourse import bass_utils
from concourse.masks import make_identity

F32 = mybir.dt.float32
F32R = mybir.dt.float32r
I32 = mybir.dt.int32


def _r(ap):
    return ap.bitcast(F32R)

# model dims (fixed by the problem)
IN_D, BOND_D, HID, OUT_D = 64, 16, 128, 64
COMB = 2 * IN_D + BOND_D + 1  # 145
KLO = COMB - 128 + 1  # 17 real rows + ones-row for b1
NCORES = 8
TILE = 512  # edges per device tile
SUB = 128  # edges per subtile (matmul K)
WIN = 64  # node-window slots per tile
PAY = 67  # payload rows: 64 msg_x + 3 pos_update
PAYP = 68  # padded (even) transposed-payload width for fp32r matmuls
# P2 psum row layout: [0:64] msg_x, [64:67] w_pos(x3), [67:83] edge_update
EU_LO, EU_HI = 67, 83
P2H = 84  # L2 output rows (83 + 1 even-pad)
GRP = 4  # tiles per output-batching group
# per-partition f32 layout of the main input stream: chi | rel | colw
MAIN_W = TILE + 12 + 4  # 528

LAST_RESULTS = None  # BassKernelResults of the most recent run (for test.py)
ACT_FUNC = mybir.ActivationFunctionType.Silu  # overridable for CoreSim tests


# ----------------------------------------------------------------------------
# host-side packing
# ----------------------------------------------------------------------------

def _plan_tiles(col_s, lo, hi):
    """Greedy tiling of sorted edge range [lo,hi): each tile takes <=TILE edges
    whose cols stay within a WIN-node window. Returns list of (pos, take, base)."""
    tiles = []
    pos = lo
    while pos < hi:
        base = int(col_s[pos])
        end = min(pos + TILE, hi)
        take = int(np.searchsorted(col_s[pos:end], base + WIN, side="left"))
        assert take > 0
        tiles.append((pos, take, base))
        pos += take
    return tiles


def _pack_core(x, pos_arr, ea_s, row_s, col_s, rel_s, dsq_s, lo, hi, T):
    """Build device input arrays for one core's sorted-edge range [lo, hi)."""
    tiles = _plan_tiles(col_s, lo, hi)
    assert len(tiles) <= T

    # per-slot source index into the sorted arrays (-1 = padding)
    idx = np.full((T, TILE), -1, dtype=np.int64)
    bases = np.zeros((T,), dtype=np.int64)
    for t, (p, take, b) in enumerate(tiles):
        idx[t, :take] = np.arange(p, p + take)
        bases[t] = b
    valid = idx >= 0
    cidx = np.where(valid, idx, 0)

    r = np.where(valid, row_s[cidx], 0)
    c = np.where(valid, col_s[cidx], 0)

    xr = np.where(valid[..., None], x[r], 0.0)  # [T, TILE, 64]
    xc = np.where(valid[..., None], x[c], 0.0)
    ea = np.where(valid[..., None], ea_s[cidx], 0.0)  # [T, TILE, 16]
    rel = np.where(valid[..., None], rel_s[cidx], 0.0)  # [T, TILE, 3]
    dsq = np.where(valid, dsq_s[cidx], 0.0)  # [T, TILE]

    nsub = TILE // SUB
    main = np.zeros((T, 128, MAIN_W), np.float32)
    main[:, 0:64, 0:TILE] = xr.transpose(0, 2, 1)
    main[:, 64:128, 0:TILE] = xc.transpose(0, 2, 1)
    # rel edge-major [T, 128, nsub, 3]
    main[:, :, TILE : TILE + 12] = (
        rel.reshape(T, nsub, SUB, 3).transpose(0, 2, 1, 3).reshape(T, 128, 12)
    )
    col_local = np.where(valid, c - bases[:, None], -1).astype(np.float32)
    main[:, :, TILE + 12 : MAIN_W] = col_local.reshape(T, nsub, SUB).transpose(
        0, 2, 1
    )

    comb_lo = np.concatenate(
        [ea.transpose(0, 2, 1), dsq[:, None, :], np.ones((T, 1, TILE))], axis=1
    ).astype(np.float32)  # [T, 18, TILE]
    # replicate at partition offsets 0/32/64 for row-group-packed matmuls
    clo_rep = np.zeros((T, 82, TILE), np.float32)
    for m in range(3):
        clo_rep[:, 32 * m : 32 * m + KLO, :] = comb_lo

    # group GRP tiles side-by-side in the free dim for big DMAs
    assert T % GRP == 0
    main_g = (
        main.reshape(T // GRP, GRP, 128, MAIN_W)
        .transpose(0, 2, 1, 3)
        .reshape(T // GRP, 128, GRP * MAIN_W)
    )
    clo_g = (
        clo_rep.reshape(T // GRP, GRP, 82, TILE)
        .transpose(0, 2, 1, 3)
        .reshape(T // GRP, 82, GRP * TILE)
    )

    return {
        "main": np.ascontiguousarray(main_g),
        "comb_lo": np.ascontiguousarray(clo_g),
    }, idx, bases


def _pack_weights(W1x, W1p, W1e, b1x, b1p, b1e, W2x, W2p, W2e, b2x, b2p, b2e):
    W1hi = np.stack([W1x[:128], W1p[:128], W1e[:128]], axis=1)  # [128, 3, HID]
    # lo rows + b1 as the ones-row weight; at 32*m partition offsets so the
    # three K=18 matmuls run concurrently in separate PE row groups
    W1lo = np.zeros((82, HID), np.float32)
    for m, (W1m, b1m) in enumerate([(W1x, b1x), (W1p, b1p), (W1e, b1e)]):
        W1lo[32 * m : 32 * m + KLO - 1] = W1m[128:]
        W1lo[32 * m + KLO - 1] = b1m
    # block-diagonal second layer: one K=384 accumulation into [84, 512]
    W2blk = np.zeros((128, 3, P2H), np.float32)
    W2blk[:, 0, 0:64] = W2x
    W2blk[:, 1, 64:67] = np.repeat(W2p, 3, axis=1)
    W2blk[:, 2, EU_LO:EU_HI] = W2e
    b2cat = np.zeros((P2H, 1), np.float32)
    b2cat[0:64, 0] = b2x
    b2cat[64:67, 0] = b2p
    b2cat[EU_LO:EU_HI, 0] = b2e
    return {
        "W1hi": np.ascontiguousarray(W1hi, np.float32),
        "W1lo": np.ascontiguousarray(W1lo, np.float32),
        "W2blk": W2blk,
        "b2cat": b2cat,
    }


# ----------------------------------------------------------------------------
# device program
# ----------------------------------------------------------------------------

def _build_bass(T, reps=1):
    nc = bacc.Bacc(trn_type="TRN2")
    nsub = TILE // SUB
    assert T % GRP == 0

    d_main = nc.dram_tensor("main", [T // GRP, 128, GRP * MAIN_W], F32R, kind="ExternalInput")
    d_clo = nc.dram_tensor("comb_lo", [T // GRP, 82, GRP * TILE], F32R, kind="ExternalInput")
    d_W1hi = nc.dram_tensor("W1hi", [128, 3, HID], F32R, kind="ExternalInput")
    d_W1lo = nc.dram_tensor("W1lo", [82, HID], F32R, kind="ExternalInput")
    d_W2blk = nc.dram_tensor("W2blk", [HID, 3, P2H], F32R, kind="ExternalInput")
    d_b2cat = nc.dram_tensor("b2cat", [P2H, 1], F32, kind="ExternalInput")

    d_win = nc.dram_tensor("winstack", [WIN, T * PAY], F32, kind="ExternalOutput")
    d_eu = nc.dram_tensor("euT", [BOND_D, T * TILE], F32, kind="ExternalOutput")

    with tile.TileContext(nc) as tc:
        with (
            tc.tile_pool(name="const", bufs=1) as constp,
            tc.tile_pool(name="io", bufs=2) as iop,
            tc.tile_pool(name="work", bufs=2) as workp,
            tc.tile_pool(name="winsb", bufs=1) as winp,
            tc.tile_pool(name="ph", bufs=2, space="PSUM") as php,
            tc.tile_pool(name="p2", bufs=1, space="PSUM") as p2p,
            tc.tile_pool(name="ptr", bufs=1, space="PSUM") as ptrp,
        ):
            # ---- constants / weights (loaded once) ----
            ident = constp.tile([128, 128], F32)
            make_identity(nc, ident[:])

            iota_i = constp.tile([128, nsub, WIN], I32)
            nc.gpsimd.iota(iota_i[:], pattern=[[0, nsub], [1, WIN]], base=0,
                           channel_multiplier=0)
            iota_f = constp.tile([128, nsub, WIN], F32R)
            nc.vector.tensor_copy(iota_f[:], iota_i[:])

            w1hi = constp.tile([128, 3, HID], F32R)
            nc.sync.dma_start(out=w1hi[:], in_=d_W1hi[:])
            w1lo = constp.tile([82, HID], F32R)
            nc.sync.dma_start(out=w1lo[:], in_=d_W1lo[:])
            w2blk = constp.tile([HID, 3, P2H], F32R)
            nc.sync.dma_start(out=w2blk[:], in_=d_W2blk[:])
            b2cat = constp.tile([P2H, 1], F32)
            nc.sync.dma_start(out=b2cat[:], in_=d_b2cat[:])

            # winstack staging buffer in SBUF (one DMA at the end); slot-major
            win_sb = winp.tile([WIN, T * PAY], F32)

            rep_ctx = tc.For_i(0, reps, 1) if reps > 1 else contextlib.nullcontext()
            with rep_ctx:
                for g in range(T // GRP):
                    agg = ptrp.tile([WIN, GRP, PAYP], F32, tag="agg")
                    main_g = iop.tile([128, GRP, MAIN_W], F32R, tag="main")
                    nc.sync.dma_start(out=main_g[:], in_=d_main[g])
                    clo_g = iop.tile([82, GRP, TILE], F32R, tag="clo")
                    nc.sync.dma_start(out=clo_g[:], in_=d_clo[g])
                    for ti in range(GRP):
                        t = g * GRP + ti
                        main = main_g[:, ti, :]
                        clo = clo_g[:, ti, :]
                        chi = main[:, 0:TILE]
                        rel = main[:, TILE : TILE + 12].rearrange(
                            "p (s k) -> p s k", k=3
                        )
                        colw = main[:, TILE + 12 : MAIN_W]

                        # ---- L1: h[m] = W1_m.T @ combT (b1 folded in lo;
                        # the three K=18 lo matmuls run in separate PE row
                        # groups concurrently) ----
                        h_ps = php.tile([128, 3, TILE], F32, tag="hps")
                        for m_ in range(3):
                            nc.tensor.matmul(
                                h_ps[:, m_, :], w1hi[:, m_, :], chi,
                                start=True, stop=False,
                            )
                        for m_ in range(3):
                            nc.tensor.matmul(
                                h_ps[:, m_, :],
                                w1lo[32 * m_ : 32 * m_ + KLO, :],
                                clo[32 * m_ : 32 * m_ + KLO, :],
                                start=False, stop=True,
                            )

                        # ---- silu -> SBUF, one op over all 3 MLPs ----
                        h_sb = workp.tile([128, 3, TILE], F32R, tag="hsb")
                        nc.scalar.activation(h_sb[:], h_ps[:], ACT_FUNC)

                        # ---- L2: one K=384 block-diag accumulation ----
                        p2 = p2p.tile([128, TILE], F32, tag="p2")
                        for c in range(3):
                            nc.tensor.matmul(
                                p2[0:P2H, :], w2blk[:, c, :], h_sb[:, c, :],
                                start=(c == 0), stop=(c == 2),
                            )

                        # ---- +b2, move to SBUF (one op) ----
                        payT = workp.tile([P2H, TILE], F32, tag="payT")
                        nc.vector.tensor_scalar(
                            out=payT[:], in0=p2[0:P2H, :],
                            scalar1=b2cat[:, 0:1], scalar2=None,
                            op0=mybir.AluOpType.add,
                        )
                        nc.sync.dma_start(
                            out=d_eu[:, t * TILE : (t + 1) * TILE],
                            in_=payT[EU_LO:EU_HI, :],
                        )

                        # ---- transpose payload to edge-major ----
                        # reuses the p2 bank: L2 results are dead once the
                        # bias moves complete (WAR deps order this correctly)
                        for s in range(nsub):
                            nc.tensor.transpose(
                                p2[:, s * PAYP : (s + 1) * PAYP],
                                payT[0:PAYP, s * SUB : (s + 1) * SUB],
                                ident[0:PAYP, 0:PAYP],
                            )
                        pay_em = workp.tile([128, nsub, PAYP], F32R, tag="payem")
                        nc.vector.tensor_copy(
                            pay_em[:],
                            p2[:, 0 : nsub * PAYP].rearrange(
                                "p (s f) -> p s f", f=PAYP
                            ),
                        )
                        # pos_update = w_pos * rel_pos
                        nc.vector.tensor_tensor(
                            out=pay_em[:, :, 64:67], in0=pay_em[:, :, 64:67],
                            in1=rel, op=mybir.AluOpType.mult,
                        )

                        # ---- S[e, slot] = (col_local[e] == slot) ----
                        S = workp.tile([128, nsub, WIN], F32R, tag="S")
                        nc.vector.tensor_tensor(
                            out=S[:], in0=iota_f[:],
                            in1=colw[:, :, None].to_broadcast([128, nsub, WIN]),
                            op=mybir.AluOpType.is_equal,
                        )

                        # ---- scatter: agg[slot, ti, feat] += S_s.T @ pay_s ----
                        for s in range(nsub):
                            nc.tensor.matmul(
                                agg[:, ti, :], S[:, s, :], pay_em[:, s, :],
                                start=(s == 0), stop=(s == nsub - 1),
                            )

                    # ---- batched outputs for the group ----
                    nc.vector.tensor_copy(
                        win_sb[:, g * GRP * PAY : (g + 1) * GRP * PAY],
                        agg[:, :, 0:PAY],
                    )

            nc.sync.dma_start(out=d_win[:], in_=win_sb[:])

    nc.finalize()
    return nc


# ----------------------------------------------------------------------------
# entry point
# ----------------------------------------------------------------------------

def host_pack(x, pos, edge_attr, edge_index,
              W1x, b1x, W2x, b2x,
              W1p, b1p, W2p, b2p,
              W1e, b1e, W2e, b2e):
    """Sort/shard/pack inputs. Returns (core_inputs, meta)."""
    x = np.asarray(x, np.float32)
    pos = np.asarray(pos, np.float32)
    edge_attr = np.asarray(edge_attr, np.float32)
    edge_index = np.asarray(edge_index)
    E = edge_index.shape[1]
    N = x.shape[0]

    row = edge_index[0].astype(np.int64)
    col = edge_index[1].astype(np.int64)
    order = np.argsort(col, kind="stable")
    row_s = row[order]
    col_s = col[order]
    ea_s = edge_attr[order]
    rel_s = pos[row_s] - pos[col_s]  # [E, 3]
    dsq_s = np.sum(rel_s * rel_s, axis=1)  # [E]

    # shard sorted edges evenly
    bounds = [E * c // NCORES for c in range(NCORES + 1)]
    plans = [
        _plan_tiles(col_s, bounds[c], bounds[c + 1]) for c in range(NCORES)
    ]
    T = max(len(p) for p in plans)
    T = (T + GRP - 1) // GRP * GRP

    core_inputs = []
    core_idx = []
    core_bases = []
    wts = _pack_weights(W1x, W1p, W1e, b1x, b1p, b1e,
                        W2x, W2p, W2e, b2x, b2p, b2e)
    for c in range(NCORES):
        inp, idx, bases = _pack_core(
            x, pos, ea_s, row_s, col_s, rel_s, dsq_s,
            bounds[c], bounds[c + 1], T,
        )
        inp.update(wts)
        core_inputs.append(inp)
        core_idx.append(idx)
        core_bases.append(bases)
    meta = {
        "N": N, "E": E, "T": T, "order": order, "plans": plans,
        "core_idx": core_idx, "core_bases": core_bases,
    }
    return core_inputs, meta


def host_merge(results, meta):
    N, E, T = meta["N"], meta["E"], meta["T"]
    order, plans = meta["order"], meta["plans"]
    core_idx, core_bases = meta["core_idx"], meta["core_bases"]

    aggX = np.zeros((N + WIN, OUT_D), np.float64)
    aggP = np.zeros((N + WIN, 3), np.float64)
    edge_update_s = np.empty((E, BOND_D), np.float32)
    for c in range(NCORES):
        out = results[c]
        win = out["winstack"].reshape(WIN, T, PAY)  # [slot, tile, feat]
        euT = out["euT"]  # [16, T*TILE]
        idx = core_idx[c]
        bases = core_bases[c]
        nt = len(plans[c])
        # overlap-add the windows
        wx = win[:, :nt, 0:64].transpose(1, 0, 2)  # [nt, WIN, 64]
        wpp = win[:, :nt, 64:67].transpose(1, 0, 2)  # [nt, WIN, 3]
        tgt = (bases[:nt, None] + np.arange(WIN)[None, :]).ravel()
        np.add.at(aggX, tgt, wx.reshape(-1, OUT_D))
        np.add.at(aggP, tgt, wpp.reshape(-1, 3))
        # edge updates back to sorted order
        eu = euT.T.reshape(T, TILE, BOND_D)
        v = idx >= 0
        edge_update_s[idx[v]] = eu[v]

    aggregated_x = aggX[:N].astype(np.float32)
    aggregated_pos = aggP[:N].astype(np.float32)
    edge_update = np.empty_like(edge_update_s)
    edge_update[order] = edge_update_s
    return aggregated_x, aggregated_pos, edge_update


def kernel(**inputs):
    global LAST_RESULTS
    core_inputs, meta = host_pack(**inputs)
    nc = _build_bass(meta["T"])
    res = bass_utils.run_bass_kernel_spmd(
        nc, core_inputs, core_ids=list(range(NCORES)),
    )
    LAST_RESULTS = res
    return host_merge(res.results, meta)
